# revision 5
# baseline (speedup 1.0000x reference)
"""Causal MHA (RoPE, 16 heads, D=1024, S=2048, B=2) on 8 trn2 NeuronCores.

Sharding: data-parallel over batch (2 groups of 4 cores) x tensor-parallel
over heads (4 heads / core). Each core computes q/k/v projections for its
256 output dims, RoPE, causal attention for its 4 heads, and a partial
output projection y_c = out_c @ Wo[:, slice].T. Host sums the 4 partials
per batch (row-parallel unshard).

v2: all matmul operands in fp16 (1 cycle/row on the PE like bf16, ~8x
better mantissa than bf16; accumulation stays fp32 in PSUM). Inputs are
converted to fp16 on the host so no on-chip staging casts are needed.
The causal mask matmuls are replaced by a post-exp multiply with a 0/1
triangular tile on the DVE. Scores are computed transposed ([keys, q])
so the attention @V matmul has q as its 512-wide free dim, and softmax
denominators come free as an extra ones-column in the V operand.
"""

import numpy as np

D_MODEL = 1024
S = 2048
NH = 16
HD = 64
THETA = 10000.0
HPC = 4          # heads per core
DPC = HPC * HD   # dims per core = 256
NG = 2           # dim groups of 128 (pairs of heads)
W = 512          # q-block width
NKO = D_MODEL // 128
NTC = S // 128   # 16 token chunks of 128

_CACHE = {}


def _build_nc():
    import concourse.bass as bass
    import concourse.tile as tile
    from concourse import bacc, mybir
    from contextlib import ExitStack

    F32 = mybir.dt.float32
    F16 = mybir.dt.float16
    AF = mybir.ActivationFunctionType
    ts = bass.ts

    nc = bacc.Bacc(None, target_bir_lowering=False)
    xT = nc.dram_tensor("xT", [D_MODEL, S], F16, kind="ExternalInput")
    wq = nc.dram_tensor("wq", [D_MODEL, DPC], F16, kind="ExternalInput")
    wk = nc.dram_tensor("wk", [D_MODEL, DPC], F16, kind="ExternalInput")
    wv = nc.dram_tensor("wv", [D_MODEL, DPC], F16, kind="ExternalInput")
    wo = nc.dram_tensor("wo", [DPC, D_MODEL], F16, kind="ExternalInput")
    coss = nc.dram_tensor("coss", [128, S], F16, kind="ExternalInput")
    sins = nc.dram_tensor("sins", [128, S], F16, kind="ExternalInput")
    pmat = nc.dram_tensor("pmat", [128, 128], F16, kind="ExternalInput")
    tri = nc.dram_tensor("tri", [128, 128], F16, kind="ExternalInput")
    y = nc.dram_tensor("y", [S, D_MODEL], F16, kind="ExternalOutput")

    with tile.TileContext(nc) as tc, ExitStack() as ctx:
        const = ctx.enter_context(tc.tile_pool(name="const", bufs=1))
        persist = ctx.enter_context(tc.tile_pool(name="persist", bufs=1))

        # persistent activations
        qT = [persist.tile([128, S], F16, name=f"qT{g}") for g in range(NG)]
        kT = [persist.tile([128, S], F16, name=f"kT{g}") for g in range(NG)]
        v_aug = persist.tile([128, NTC, HPC * (HD + 1)], F16, name="v_aug")
        out_cT = [persist.tile([128, S], F16, name=f"out_cT{g}")
                  for g in range(NG)]
        wo_r = persist.tile([128, NG, D_MODEL], F16, name="wo_r")

        # softmax-denominator ones columns of v_aug, set once
        nc.gpsimd.memset(v_aug[:, :, HD::HD + 1], 1.0)

        # ---- phase 1: QKV + RoPE (x streamed in 4 quarters) -----------
        with nc.named_scope("qkv"), \
             tc.tile_pool(name="qkvw", bufs=1) as wpool, \
             tc.tile_pool(name="qkv", bufs=3) as qkv_pool, \
             tc.tile_pool(name="xtr", bufs=2) as xt_pool, \
             tc.tile_pool(name="ps1v", bufs=2, space="PSUM") as ps1v, \
             tc.tile_pool(name="ps1qk", bufs=4, space="PSUM") as ps1qk, \
             tc.tile_pool(name="ps1p", bufs=2, space="PSUM") as ps1p:

            def load_w(name, dram, width):
                t = wpool.tile([128, NKO, width], F16, name=name + "_r")
                for ko in range(NKO):
                    nc.sync.dma_start(t[:, ko], dram.ap()[ts(ko, 128), :])
                return t

            def load_x_quarter(hf):
                xr = xt_pool.tile([128, NKO, W], F16, tag="xT_r", name="xT_r")
                for ko in range(NKO):
                    nc.sync.dma_start(xr[:, ko], xT.ap()[ts(ko, 128), ts(hf, W)])
                return xr

            wv_r = load_w("wv", wv, DPC)
            xquart = load_x_quarter(0)
            wq_r = load_w("wq", wq, DPC)
            wk_r = load_w("wk", wk, DPC)

            pm_r = const.tile([128, 128], F16)
            nc.sync.dma_start(pm_r[:], pmat.ap())
            tri_r = const.tile([128, 128], F16)
            nc.sync.dma_start(tri_r[:], tri.ap())
            cs_t = const.tile([128, S], F16)
            sn_t = const.tile([128, S], F16)
            for j4 in range(4):
                nc.sync.dma_start(cs_t[:, ts(j4, W)], coss.ap()[:, ts(j4, W)])
                nc.sync.dma_start(sn_t[:, ts(j4, W)], sins.ap()[:, ts(j4, W)])
            for g2 in range(NG):
                nc.sync.dma_start(wo_r[:, g2], wo.ap()[ts(g2, 128), :])

            def do_v(xT_r, hf):
                for tl in range(W // 128):
                    tcN = hf * (W // 128) + tl
                    psv = ps1v.tile([128, DPC], F32, tag="psv", name="psv")
                    for ko in range(NKO):
                        nc.tensor.matmul(psv[:], xT_r[:, ko, ts(tl, 128)],
                                         wv_r[:, ko],
                                         start=(ko == 0), stop=(ko == NKO - 1))
                    nc.vector.tensor_copy(
                        v_aug[:, tcN].rearrange("p (h c) -> p h c",
                                                h=HPC)[:, :, 0:HD],
                        psv[:].rearrange("p (h c) -> p h c", h=HPC))

            def do_qk(xT_r, hf):
                for g in range(NG):
                    psq = ps1qk.tile([128, W], F32, tag="psqk", name="psq")
                    for ko in range(NKO):
                        nc.tensor.matmul(
                            psq[:], wq_r[:, ko, ts(g, 128)], xT_r[:, ko],
                            start=(ko == 0), stop=(ko == NKO - 1))
                    rawq = qkv_pool.tile([128, W], F16, tag="rawq",
                                         name="rawq")
                    nc.scalar.copy(rawq[:], psq[:])
                    psk = ps1qk.tile([128, W], F32, tag="psqk", name="psk")
                    for ko in range(NKO):
                        nc.tensor.matmul(
                            psk[:], wk_r[:, ko, ts(g, 128)], xT_r[:, ko],
                            start=(ko == 0), stop=(ko == NKO - 1))
                    rawk = qkv_pool.tile([128, W], F16, tag="rawk",
                                         name="rawk")
                    nc.scalar.copy(rawk[:], psk[:])
                    for nm, raw, ps, dst in (("q", rawq, psq, qT[g]),
                                             ("k", rawk, psk, kT[g])):
                        psp = ps1p.tile([128, W], F32, tag="psp", name="psp")
                        nc.tensor.matmul(psp[:], pm_r[:], raw[:],
                                         start=True, stop=True)
                        t1 = qkv_pool.tile([128, W], F16, tag=f"t1{nm}",
                                           name="t1")
                        nc.vector.tensor_tensor(t1[:], raw[:],
                                                cs_t[:, ts(hf, W)],
                                                mybir.AluOpType.mult)
                        t2 = qkv_pool.tile([128, W], F16, tag=f"t2{nm}",
                                           name="t2")
                        nc.vector.tensor_tensor(t2[:], psp[:],
                                                sn_t[:, ts(hf, W)],
                                                mybir.AluOpType.mult)
                        nc.gpsimd.tensor_tensor(dst[:, ts(hf, W)],
                                                t1[:], t2[:],
                                                mybir.AluOpType.add)

            for hf in range(4):
                xT_r = xquart
                if hf < 3:
                    xquart = load_x_quarter(hf + 1)
                    do_v(xT_r, hf)
                    do_qk(xT_r, hf)
                else:
                    # last quarter: q/k first so their scalar/DVE/gpsimd
                    # consumers drain while the PE runs the v matmuls --
                    # keeps the phase-boundary bubble small.
                    do_qk(xT_r, hf)
                    do_v(xT_r, hf)

        # ---- phase 2: attention (+ interleaved output projection) -----
        with nc.named_scope("attn"), \
             tc.tile_pool(name="att", bufs=6) as att_pool, \
             tc.tile_pool(name="norm", bufs=3) as norm_pool, \
             tc.tile_pool(name="ps2", bufs=2, space="PSUM") as ps2, \
             tc.tile_pool(name="ps2av", bufs=1, space="PSUM") as ps2av:
            pending = []

            def emit_oproj(tcN):
                ysb = norm_pool.tile([128, D_MODEL], F16, tag="ysb",
                                     name="ysb")
                for e2 in range(2):
                    psy = ps2.tile([128, W], F32, tag="sc", name="psy")
                    for g in range(NG):
                        nc.tensor.matmul(psy[:], out_cT[g][:, ts(tcN, 128)],
                                         wo_r[:, g, ts(e2, W)],
                                         start=(g == 0), stop=(g == NG - 1),
                                         skip_group_check=True)
                    nc.vector.tensor_copy(ysb[:, ts(e2, W)], psy[:])
                nc.sync.dma_start(y.ap()[ts(tcN, 128), :], ysb[:])

            LAG = 2
            for qb in range(S // W):
                av = [ps2av.tile([HD + 1, W], F32, tag=f"av{hh}",
                                 name=f"av{hh}") for hh in range(4)]
                nkb = (qb + 1) * (W // 128)
                attq = []

                def emit_av(entry, nkb=nkb, av=av):
                    kb, cs0, atts = entry
                    for g in range(NG):
                        for h in range(2):
                            hh = 2 * g + h
                            nc.tensor.matmul(
                                av[hh][:, cs0:],
                                v_aug[:, kb, hh * (HD + 1):
                                      (hh + 1) * (HD + 1)],
                                atts[g][:, h * W + cs0:(h + 1) * W],
                                start=(kb == 0), stop=(kb == nkb - 1),
                                skip_group_check=True)

                for kb in range(nkb):
                    if kb >= 3 and pending:
                        emit_oproj(pending.pop(0))
                    cs0 = max(0, kb * 128 - qb * W)
                    diag = kb * 128 >= qb * W
                    atts = []
                    for g in range(NG):
                        sc = ps2.tile([128, 2 * W], F32, tag="sc", name="sc")
                        for h in range(2):
                            nc.tensor.matmul(
                                sc[:, h * W + cs0:(h + 1) * W],
                                kT[g][ts(h, HD), ts(kb, 128)],
                                qT[g][ts(h, HD), qb * W + cs0:(qb + 1) * W],
                                start=True, stop=True,
                                skip_group_check=True)
                        att = att_pool.tile([128, 2 * W], F16, tag="attw",
                                            name="att")
                        scv = sc[:].rearrange("p (h w) -> p h w", h=2)
                        atv = att[:].rearrange("p (h w) -> p h w", h=2)
                        nc.scalar.activation(atv[:, :, cs0:], scv[:, :, cs0:],
                                             AF.Exp, scale=1.0 / np.sqrt(HD))
                        if diag:
                            for h in range(2):
                                dslc = slice(h * W + cs0, h * W + cs0 + 128)
                                nc.vector.tensor_tensor(
                                    att[:, dslc], att[:, dslc], tri_r[:],
                                    mybir.AluOpType.mult)
                        atts.append(att)
                    attq.append((kb, cs0, atts))
                    if len(attq) > LAG:
                        emit_av(attq.pop(0))
                while attq:
                    emit_av(attq.pop(0))
                rss = []
                for hh in range(4):
                    rs = norm_pool.tile([1, W], F32, tag=f"rs{hh}", name="rs")
                    nc.vector.tensor_copy(rs[:], av[hh][HD:HD + 1, :])
                    rss.append(rs)
                recs = []
                for hh in range(4):
                    rec = norm_pool.tile([1, W], F32, tag=f"rec{hh}",
                                         name="rec")
                    nc.vector.reciprocal_approx_fast(rec[:], rss[hh][:])
                    recs.append(rec)
                rbs = []
                for hh in range(4):
                    rb = norm_pool.tile([HD, W], F32, tag=f"rb{hh}", name="rb")
                    nc.gpsimd.partition_broadcast(rb[:], recs[hh][:])
                    rbs.append(rb)
                for hh in range(4):
                    g, h = divmod(hh, 2)
                    nc.vector.tensor_tensor(
                        out_cT[g][ts(h, HD), ts(qb, W)],
                        av[hh][0:HD, :], rbs[hh][:], mybir.AluOpType.mult)
                pending.extend(qb * (W // 128) + tl for tl in range(W // 128))
            for tcN in pending:
                emit_oproj(tcN)

    nc.compile()
    return nc


def _host_inputs():
    d = HD
    inv_freq = THETA ** (-np.arange(0, d, 2, dtype=np.float64) / d)  # [32]
    t = np.arange(S, dtype=np.float64)
    ang = t[None, :] * inv_freq[:, None]          # [32, S]
    C64 = np.repeat(np.cos(ang), 2, axis=0)       # [64, S] per-dim cos
    S64 = np.repeat(np.sin(ang), 2, axis=0).copy()
    S64[0::2] *= -1.0                             # even dims: -sin
    C = np.tile(C64, (2, 1)).astype(np.float16)   # [128, S] two heads
    Sg = np.tile(S64, (2, 1)).astype(np.float16)

    P = np.zeros((128, 128), np.float16)
    idx = np.arange(128)
    P[idx ^ 1, idx] = 1.0

    # tri[k, q] = 1 where q >= k (causal keep), applied post-exp
    T = (np.arange(128)[None, :] >= np.arange(128)[:, None]
         ).astype(np.float16)
    return C, Sg, P, T


def kernel(x, Wq, Wk, Wv, Wo):
    from concourse.bass_utils import run_bass_kernel_spmd

    x = np.asarray(x, np.float32)
    Wq = np.asarray(Wq, np.float32)
    Wk = np.asarray(Wk, np.float32)
    Wv = np.asarray(Wv, np.float32)
    Wo = np.asarray(Wo, np.float32)
    B = x.shape[0]

    if "nc" not in _CACHE:
        _CACHE["nc"] = _build_nc()
    nc = _CACHE["nc"]

    C, Sg, P, T = _host_inputs()
    xTb = [np.ascontiguousarray(x[b].T).astype(np.float16) for b in range(B)]
    in_maps = []
    for c in range(8):
        b, hq = divmod(c, 4)
        sl = slice(hq * DPC, (hq + 1) * DPC)
        in_maps.append({
            "xT": xTb[b],
            "wq": np.ascontiguousarray(Wq[sl, :].T).astype(np.float16),
            "wk": np.ascontiguousarray(Wk[sl, :].T).astype(np.float16),
            "wv": np.ascontiguousarray(Wv[sl, :].T).astype(np.float16),
            "wo": np.ascontiguousarray(Wo[:, sl].T).astype(np.float16),
            "coss": C, "sins": Sg, "pmat": P, "tri": T,
        })

    res = run_bass_kernel_spmd(nc, in_maps, list(range(8)),
                               **_CACHE.get("runkw", {}))
    _CACHE["last_res"] = res
    out = np.zeros((B, S, D_MODEL), np.float32)
    for c in range(8):
        b = c // 4
        out[b] += res.results[c]["y"].astype(np.float32)
    return out


# revision 6
# speedup vs baseline: 1.1117x; 1.1117x over previous
"""Causal MHA (RoPE, 16 heads, D=1024, S=2048, B=2) on 8 trn2 NeuronCores.

Sharding: batch (2 groups of 4 cores) x tensor-parallel heads (4/core).
v4: fp16 matmuls; qb0/qb1 attention scores+softmax-exp are interleaved
into phase 1 (QKV) so the Scalar engine's exp work overlaps the QKV
matmuls; phase 2 starts with a PE-only AV prologue over the stored att
tiles while early qb2 score/exp units keep the Scalar engine fed.
"""

import numpy as np

D_MODEL = 1024
S = 2048
NH = 16
HD = 64
THETA = 10000.0
HPC = 4          # heads per core
DPC = HPC * HD   # dims per core = 256
NG = 2           # dim groups of 128 (pairs of heads)
W = 512          # q-block width
NKO = D_MODEL // 128
NTC = S // 128   # 16 token chunks of 128

_CACHE = {}


def _build_nc():
    import concourse.bass as bass
    import concourse.tile as tile
    from concourse import bacc, mybir
    from contextlib import ExitStack

    F32 = mybir.dt.float32
    F16 = mybir.dt.float16
    AF = mybir.ActivationFunctionType
    ts = bass.ts
    MUL = mybir.AluOpType.mult
    SCALE = 1.0 / np.sqrt(HD)

    nc = bacc.Bacc(None, target_bir_lowering=False)
    xT = nc.dram_tensor("xT", [D_MODEL, S], F16, kind="ExternalInput")
    wq = nc.dram_tensor("wq", [D_MODEL, DPC], F16, kind="ExternalInput")
    wk = nc.dram_tensor("wk", [D_MODEL, DPC], F16, kind="ExternalInput")
    wv = nc.dram_tensor("wv", [D_MODEL, DPC], F16, kind="ExternalInput")
    wo = nc.dram_tensor("wo", [DPC, D_MODEL], F16, kind="ExternalInput")
    coss = nc.dram_tensor("coss", [128, S], F16, kind="ExternalInput")
    sins = nc.dram_tensor("sins", [128, S], F16, kind="ExternalInput")
    pmat = nc.dram_tensor("pmat", [128, 128], F16, kind="ExternalInput")
    tri = nc.dram_tensor("tri", [128, 128], F16, kind="ExternalInput")
    y = nc.dram_tensor("y", [S, D_MODEL], F16, kind="ExternalOutput")

    with tile.TileContext(nc) as tc, ExitStack() as ctx:
        const = ctx.enter_context(tc.tile_pool(name="const", bufs=1))
        persist = ctx.enter_context(tc.tile_pool(name="persist", bufs=1))

        qT = [persist.tile([128, S], F16, name=f"qT{g}") for g in range(NG)]
        kT = [persist.tile([128, S], F16, name=f"kT{g}") for g in range(NG)]
        v_aug = persist.tile([128, NTC, HPC * (HD + 1)], F16, name="v_aug")
        out_cT = [persist.tile([128, S], F16, name=f"out_cT{g}")
                  for g in range(NG)]
        wo_r = persist.tile([128, NG, D_MODEL], F16, name="wo_r")
        # stored per-head att tiles for qb0/qb1 (exp'd during phase 1)
        att01 = {}
        for q01 in range(2):
            for kb in range((q01 + 1) * 4):
                for g in range(NG):
                    for h in range(2):
                        att01[(q01, kb, g, h)] = persist.tile(
                            [128, W], F16, name=f"a{q01}_{kb}_{g}_{h}")

        nc.gpsimd.memset(v_aug[:, :, HD::HD + 1], 1.0)

        pm_r = const.tile([128, 128], F16)
        tri_r = const.tile([128, 128], F16)
        cs_t = const.tile([128, S], F16)
        sn_t = const.tile([128, S], F16)

        # ---- phase 1: QKV + RoPE + qb0/qb1 scores+exp -----------------
        with nc.named_scope("qkv"), \
             tc.tile_pool(name="qkvw", bufs=1) as wpool, \
             tc.tile_pool(name="qkv", bufs=3) as qkv_pool, \
             tc.tile_pool(name="xtr", bufs=2) as xt_pool, \
             tc.tile_pool(name="ps1v", bufs=2, space="PSUM") as ps1v, \
             tc.tile_pool(name="ps1qk", bufs=2, space="PSUM") as ps1qk, \
             tc.tile_pool(name="ps1p", bufs=2, space="PSUM") as ps1p, \
             tc.tile_pool(name="sc1", bufs=2, space="PSUM") as sc1:

            tasks = []

            def emit_unit(qb, kb, g, h):
                cs0 = max(0, kb * 128 - qb * W)
                diag = kb * 128 >= qb * W
                sc = sc1.tile([128, W], F32, tag="sc1", name="sc1")
                nc.tensor.matmul(
                    sc[:, cs0:], kT[g][ts(h, HD), ts(kb, 128)],
                    qT[g][ts(h, HD), qb * W + cs0:(qb + 1) * W],
                    start=True, stop=True, skip_group_check=True)
                ath = att01[(qb, kb, g, h)]
                nc.scalar.activation(ath[:, cs0:], sc[:, cs0:], AF.Exp,
                                     scale=SCALE)
                if diag:
                    nc.vector.tensor_tensor(ath[:, cs0:cs0 + 128],
                                            ath[:, cs0:cs0 + 128],
                                            tri_r[:], MUL)

            def filler(budget=2):
                for _ in range(budget):
                    if tasks:
                        emit_unit(*tasks.pop(0))

            def load_w(name, dram, width):
                t = wpool.tile([128, NKO, width], F16, name=name + "_r")
                for ko in range(NKO):
                    nc.sync.dma_start(t[:, ko], dram.ap()[ts(ko, 128), :])
                return t

            def load_x_quarter(hf):
                xr = xt_pool.tile([128, NKO, W], F16, tag="xT_r", name="xT_r")
                for ko in range(NKO):
                    nc.sync.dma_start(xr[:, ko], xT.ap()[ts(ko, 128), ts(hf, W)])
                return xr

            wv_r = load_w("wv", wv, DPC)
            xquart = load_x_quarter(0)
            wq_r = load_w("wq", wq, DPC)
            wk_r = load_w("wk", wk, DPC)

            nc.sync.dma_start(pm_r[:], pmat.ap())
            nc.sync.dma_start(tri_r[:], tri.ap())
            for j4 in range(4):
                nc.sync.dma_start(cs_t[:, ts(j4, W)], coss.ap()[:, ts(j4, W)])
                nc.sync.dma_start(sn_t[:, ts(j4, W)], sins.ap()[:, ts(j4, W)])
            for g2 in range(NG):
                nc.sync.dma_start(wo_r[:, g2], wo.ap()[ts(g2, 128), :])

            def do_v(xT_r, hf):
                for tl in range(W // 128):
                    tcN = hf * (W // 128) + tl
                    psv = ps1v.tile([128, DPC], F32, tag="psv", name="psv")
                    for ko in range(NKO):
                        nc.tensor.matmul(psv[:], xT_r[:, ko, ts(tl, 128)],
                                         wv_r[:, ko],
                                         start=(ko == 0), stop=(ko == NKO - 1))
                    nc.vector.tensor_copy(
                        v_aug[:, tcN].rearrange("p (h c) -> p h c",
                                                h=HPC)[:, :, 0:HD],
                        psv[:].rearrange("p (h c) -> p h c", h=HPC))
                    filler()

            def do_qk(xT_r, hf):
                for g in range(NG):
                    psq = ps1qk.tile([128, W], F32, tag="psqk", name="psq")
                    for ko in range(NKO):
                        nc.tensor.matmul(
                            psq[:], wq_r[:, ko, ts(g, 128)], xT_r[:, ko],
                            start=(ko == 0), stop=(ko == NKO - 1))
                    rawq = qkv_pool.tile([128, W], F16, tag="rawq",
                                         name="rawq")
                    nc.scalar.copy(rawq[:], psq[:])
                    psk = ps1qk.tile([128, W], F32, tag="psqk", name="psk")
                    for ko in range(NKO):
                        nc.tensor.matmul(
                            psk[:], wk_r[:, ko, ts(g, 128)], xT_r[:, ko],
                            start=(ko == 0), stop=(ko == NKO - 1))
                    rawk = qkv_pool.tile([128, W], F16, tag="rawk",
                                         name="rawk")
                    nc.scalar.copy(rawk[:], psk[:])
                    for nm, raw, dst in (("q", rawq, qT[g]),
                                         ("k", rawk, kT[g])):
                        psp = ps1p.tile([128, W], F32, tag="psp", name="psp")
                        nc.tensor.matmul(psp[:], pm_r[:], raw[:],
                                         start=True, stop=True)
                        t1 = qkv_pool.tile([128, W], F16, tag=f"t1{nm}",
                                           name="t1")
                        nc.vector.tensor_tensor(t1[:], raw[:],
                                                cs_t[:, ts(hf, W)], MUL)
                        t2 = qkv_pool.tile([128, W], F16, tag=f"t2{nm}",
                                           name="t2")
                        nc.vector.tensor_tensor(t2[:], psp[:],
                                                sn_t[:, ts(hf, W)], MUL)
                        nc.gpsimd.tensor_tensor(dst[:, ts(hf, W)],
                                                t1[:], t2[:],
                                                mybir.AluOpType.add)
                    filler()

            for hf in range(4):
                xT_r = xquart
                if hf < 3:
                    xquart = load_x_quarter(hf + 1)
                    do_v(xT_r, hf)
                    do_qk(xT_r, hf)
                else:
                    do_qk(xT_r, hf)
                    do_v(xT_r, hf)
                if hf == 0:
                    tasks.extend((0, kb, g, h) for kb in range(4)
                                 for g in range(NG) for h in range(2))
                elif hf == 1:
                    tasks.extend((1, kb, g, h) for kb in range(8)
                                 for g in range(NG) for h in range(2))
            while tasks:
                emit_unit(*tasks.pop(0))

        # ---- phase 2: attention (+ interleaved output projection) -----
        with nc.named_scope("attn"), \
             tc.tile_pool(name="att", bufs=12) as att_pool, \
             tc.tile_pool(name="norm", bufs=3) as norm_pool, \
             tc.tile_pool(name="ps2", bufs=2, space="PSUM") as ps2, \
             tc.tile_pool(name="ps2av", bufs=1, space="PSUM") as ps2av:
            pending = []

            def emit_oproj(tcN):
                ysb = norm_pool.tile([128, D_MODEL], F16, tag="ysb",
                                     name="ysb")
                for e2 in range(2):
                    psy = ps2.tile([128, W], F32, tag="sc", name="psy")
                    for g in range(NG):
                        nc.tensor.matmul(psy[:], out_cT[g][:, ts(tcN, 128)],
                                         wo_r[:, g, ts(e2, W)],
                                         start=(g == 0), stop=(g == NG - 1),
                                         skip_group_check=True)
                    nc.vector.tensor_copy(ysb[:, ts(e2, W)], psy[:])
                nc.sync.dma_start(y.ap()[ts(tcN, 128), :], ysb[:])

            def make_entry(qb, kb):
                """scores + exp (+tri) for one kb of qb (two-head tiles)."""
                cs0 = max(0, kb * 128 - qb * W)
                diag = kb * 128 >= qb * W
                atts = []
                for g in range(NG):
                    sc = ps2.tile([128, 2 * W], F32, tag="sc", name="sc")
                    for h in range(2):
                        nc.tensor.matmul(
                            sc[:, h * W + cs0:(h + 1) * W],
                            kT[g][ts(h, HD), ts(kb, 128)],
                            qT[g][ts(h, HD), qb * W + cs0:(qb + 1) * W],
                            start=True, stop=True, skip_group_check=True)
                    att = att_pool.tile([128, 2 * W], F16, tag="attw",
                                        name="att")
                    scv = sc[:].rearrange("p (h w) -> p h w", h=2)
                    atv = att[:].rearrange("p (h w) -> p h w", h=2)
                    nc.scalar.activation(atv[:, :, cs0:], scv[:, :, cs0:],
                                         AF.Exp, scale=SCALE)
                    if diag:
                        for h in range(2):
                            dslc = slice(h * W + cs0, h * W + cs0 + 128)
                            nc.vector.tensor_tensor(att[:, dslc], att[:, dslc],
                                                    tri_r[:], MUL)
                    atts.append(att)
                return (kb, cs0, atts)

            def normalize(av, qb):
                rss = []
                for hh in range(4):
                    rs = norm_pool.tile([1, W], F32, tag=f"rs{hh}", name="rs")
                    nc.vector.tensor_copy(rs[:], av[hh][HD:HD + 1, :])
                    rss.append(rs)
                recs = []
                for hh in range(4):
                    rec = norm_pool.tile([1, W], F32, tag=f"rec{hh}",
                                         name="rec")
                    nc.vector.reciprocal_approx_fast(rec[:], rss[hh][:])
                    recs.append(rec)
                rbs = []
                for hh in range(4):
                    rb = norm_pool.tile([HD, W], F32, tag=f"rb{hh}", name="rb")
                    nc.gpsimd.partition_broadcast(rb[:], recs[hh][:])
                    rbs.append(rb)
                for hh in range(4):
                    g, h = divmod(hh, 2)
                    nc.vector.tensor_tensor(
                        out_cT[g][ts(h, HD), ts(qb, W)],
                        av[hh][0:HD, :], rbs[hh][:], MUL)

            early = []       # pre-built (kb, cs0, atts) entries for qb=2
            NEARLY = 3
            LAG = 2

            for qb in range(S // W):
                av = [ps2av.tile([HD + 1, W], F32, tag=f"av{hh}",
                                 name=f"av{hh}") for hh in range(4)]
                nkb = (qb + 1) * (W // 128)

                if qb < 2:
                    # AV prologue from stored att01 tiles (PE-only), with
                    # early qb2 score/exp units to keep Scalar busy.
                    for kb in range(nkb):
                        if kb % 3 == 1 and len(early) < NEARLY:
                            early.append(make_entry(2, len(early)))
                        cs0 = max(0, kb * 128 - qb * W)
                        for g in range(NG):
                            for h in range(2):
                                hh = 2 * g + h
                                nc.tensor.matmul(
                                    av[hh][:, cs0:],
                                    v_aug[:, kb, hh * (HD + 1):
                                          (hh + 1) * (HD + 1)],
                                    att01[(qb, kb, g, h)][:, cs0:],
                                    start=(kb == 0), stop=(kb == nkb - 1),
                                    skip_group_check=True)
                        if kb >= 3 and pending:
                            emit_oproj(pending.pop(0))
                    normalize(av, qb)
                    pending.extend(qb * (W // 128) + tl
                                   for tl in range(W // 128))
                    continue

                attq = list(early) if qb == 2 else []
                early = []
                start_kb = len(attq)

                def emit_av(entry, nkb=nkb, av=av):
                    kb, cs0, atts = entry
                    for g in range(NG):
                        for h in range(2):
                            hh = 2 * g + h
                            nc.tensor.matmul(
                                av[hh][:, cs0:],
                                v_aug[:, kb, hh * (HD + 1):
                                      (hh + 1) * (HD + 1)],
                                atts[g][:, h * W + cs0:(h + 1) * W],
                                start=(kb == 0), stop=(kb == nkb - 1),
                                skip_group_check=True)

                for kb in range(start_kb, nkb):
                    if kb >= 3 and pending:
                        emit_oproj(pending.pop(0))
                    attq.append(make_entry(qb, kb))
                    if len(attq) > LAG:
                        emit_av(attq.pop(0))
                while attq:
                    emit_av(attq.pop(0))
                normalize(av, qb)
                pending.extend(qb * (W // 128) + tl for tl in range(W // 128))
            for tcN in pending:
                emit_oproj(tcN)

    nc.compile()
    return nc


def _host_inputs():
    d = HD
    inv_freq = THETA ** (-np.arange(0, d, 2, dtype=np.float64) / d)  # [32]
    t = np.arange(S, dtype=np.float64)
    ang = t[None, :] * inv_freq[:, None]          # [32, S]
    C64 = np.repeat(np.cos(ang), 2, axis=0)       # [64, S] per-dim cos
    S64 = np.repeat(np.sin(ang), 2, axis=0).copy()
    S64[0::2] *= -1.0                             # even dims: -sin
    C = np.tile(C64, (2, 1)).astype(np.float16)   # [128, S] two heads
    Sg = np.tile(S64, (2, 1)).astype(np.float16)

    P = np.zeros((128, 128), np.float16)
    idx = np.arange(128)
    P[idx ^ 1, idx] = 1.0

    # tri[k, q] = 1 where q >= k (causal keep), applied post-exp
    T = (np.arange(128)[None, :] >= np.arange(128)[:, None]
         ).astype(np.float16)
    return C, Sg, P, T


def kernel(x, Wq, Wk, Wv, Wo):
    from concourse.bass_utils import run_bass_kernel_spmd

    x = np.asarray(x, np.float32)
    Wq = np.asarray(Wq, np.float32)
    Wk = np.asarray(Wk, np.float32)
    Wv = np.asarray(Wv, np.float32)
    Wo = np.asarray(Wo, np.float32)
    B = x.shape[0]

    if "nc" not in _CACHE:
        _CACHE["nc"] = _build_nc()
    nc = _CACHE["nc"]

    C, Sg, P, T = _host_inputs()
    xTb = [np.ascontiguousarray(x[b].T).astype(np.float16) for b in range(B)]
    in_maps = []
    for c in range(8):
        b, hq = divmod(c, 4)
        sl = slice(hq * DPC, (hq + 1) * DPC)
        in_maps.append({
            "xT": xTb[b],
            "wq": np.ascontiguousarray(Wq[sl, :].T).astype(np.float16),
            "wk": np.ascontiguousarray(Wk[sl, :].T).astype(np.float16),
            "wv": np.ascontiguousarray(Wv[sl, :].T).astype(np.float16),
            "wo": np.ascontiguousarray(Wo[:, sl].T).astype(np.float16),
            "coss": C, "sins": Sg, "pmat": P, "tri": T,
        })

    res = run_bass_kernel_spmd(nc, in_maps, list(range(8)),
                               **_CACHE.get("runkw", {}))
    _CACHE["last_res"] = res
    out = np.zeros((B, S, D_MODEL), np.float32)
    for c in range(8):
        b = c // 4
        out[b] += res.results[c]["y"].astype(np.float32)
    return out


# revision 16
# speedup vs baseline: 1.2888x; 1.1593x over previous
"""Causal MHA (RoPE, 16 heads, D=1024, S=2048, B=2) on 8 trn2 NeuronCores.

Sharding: batch (2 groups of 4 cores) x tensor-parallel heads (4/core).
v4: fp16 matmuls; qb0/qb1 attention scores+softmax-exp are interleaved
into phase 1 (QKV) so the Scalar engine's exp work overlaps the QKV
matmuls; phase 2 starts with a PE-only AV prologue over the stored att
tiles while early qb2 score/exp units keep the Scalar engine fed.
"""

import numpy as np

D_MODEL = 1024
S = 2048
NH = 16
HD = 64
THETA = 10000.0
HPC = 4          # heads per core
DPC = HPC * HD   # dims per core = 256
NG = 2           # dim groups of 128 (pairs of heads)
W = 512          # q-block width
NKO = D_MODEL // 128
NTC = S // 128   # 16 token chunks of 128

_CACHE = {}


def _build_nc():
    import concourse.bass as bass
    import concourse.tile as tile
    from concourse import bacc, mybir
    from contextlib import ExitStack

    F32 = mybir.dt.float32
    F16 = mybir.dt.float16
    AF = mybir.ActivationFunctionType
    ts = bass.ts
    MUL = mybir.AluOpType.mult
    SCALE = 1.0 / np.sqrt(HD)

    nc = bacc.Bacc(None, target_bir_lowering=False)
    xT = nc.dram_tensor("xT", [D_MODEL, S], F16, kind="ExternalInput")
    wq = nc.dram_tensor("wq", [D_MODEL, DPC], F16, kind="ExternalInput")
    wk = nc.dram_tensor("wk", [D_MODEL, DPC], F16, kind="ExternalInput")
    wv = nc.dram_tensor("wv", [D_MODEL, DPC], F16, kind="ExternalInput")
    wo = nc.dram_tensor("wo", [DPC, D_MODEL], F16, kind="ExternalInput")
    coss = nc.dram_tensor("coss", [128, S], F16, kind="ExternalInput")
    sins = nc.dram_tensor("sins", [128, S], F16, kind="ExternalInput")
    pmat = nc.dram_tensor("pmat", [128, 128], F16, kind="ExternalInput")
    tri = nc.dram_tensor("tri", [128, 128], F16, kind="ExternalInput")
    y = nc.dram_tensor("y", [S, D_MODEL], F16, kind="ExternalOutput")

    with tile.TileContext(nc) as tc, ExitStack() as ctx:
        const = ctx.enter_context(tc.tile_pool(name="const", bufs=1))
        persist = ctx.enter_context(tc.tile_pool(name="persist", bufs=1))

        qT = [persist.tile([128, S], F16, name=f"qT{g}") for g in range(NG)]
        kT = [persist.tile([128, S], F16, name=f"kT{g}") for g in range(NG)]
        v_aug = persist.tile([128, NTC, HPC * (HD + 1)], F16, name="v_aug")
        out_cT = [persist.tile([128, S], F16, name=f"out_cT{g}")
                  for g in range(NG)]
        wo_r = persist.tile([128, NG, D_MODEL], F16, name="wo_r")
        # stored per-head att tiles for qb0/qb1 (exp'd during phase 1)
        att01 = {}
        for q01 in range(2):
            for kb in range((q01 + 1) * 4):
                for g in range(NG):
                    for h in range(2):
                        att01[(q01, kb, g, h)] = persist.tile(
                            [128, W], F16, name=f"a{q01}_{kb}_{g}_{h}")

        nc.gpsimd.memset(v_aug[:, :, HD::HD + 1], 1.0)
        # pre-warm the gpsimd partition_broadcast microcode library off the
        # critical path (first real use is at qb0's softmax normalize)
        warm_in = const.tile([1, 8], F32)
        warm_out = const.tile([2, 8], F32)
        nc.gpsimd.memset(warm_in[:], 1.0)
        nc.gpsimd.partition_broadcast(warm_out[:], warm_in[:])

        pm_r = const.tile([128, 128], F16)
        tri_r = const.tile([128, 128], F16)
        cs_t = const.tile([128, S], F16)
        sn_t = const.tile([128, S], F16)

        # ---- phase 1: QKV + RoPE + qb0/qb1 scores+exp -----------------
        with nc.named_scope("qkv"), \
             tc.tile_pool(name="qkvw", bufs=1) as wpool, \
             tc.tile_pool(name="qkv", bufs=3) as qkv_pool, \
             tc.tile_pool(name="xtr", bufs=2) as xt_pool, \
             tc.tile_pool(name="ps1v", bufs=2, space="PSUM") as ps1v, \
             tc.tile_pool(name="ps1qk", bufs=2, space="PSUM") as ps1qk, \
             tc.tile_pool(name="ps1p", bufs=2, space="PSUM") as ps1p, \
             tc.tile_pool(name="sc1", bufs=2, space="PSUM") as sc1:

            tasks = []

            def emit_unit(qb, kb, g, h):
                cs0 = max(0, kb * 128 - qb * W)
                diag = kb * 128 >= qb * W
                sc = sc1.tile([128, W], F32, tag="sc1", name="sc1")
                nc.tensor.matmul(
                    sc[:, cs0:], kT[g][ts(h, HD), ts(kb, 128)],
                    qT[g][ts(h, HD), qb * W + cs0:(qb + 1) * W],
                    start=True, stop=True, skip_group_check=True)
                ath = att01[(qb, kb, g, h)]
                nc.scalar.activation(ath[:, cs0:], sc[:, cs0:], AF.Exp,
                                     scale=SCALE)
                if diag:
                    nc.vector.tensor_tensor(ath[:, cs0:cs0 + 128],
                                            ath[:, cs0:cs0 + 128],
                                            tri_r[:], MUL)

            def filler(budget=2):
                for _ in range(budget):
                    if tasks:
                        emit_unit(*tasks.pop(0))

            def load_w(name, dram, width):
                t = wpool.tile([128, NKO, width], F16, name=name + "_r")
                nc.sync.dma_start(
                    t[:], dram.ap().rearrange("(ko p) c -> p ko c", p=128))
                return t

            xT_v = xT.ap().rearrange("(ko p) s -> p ko s", p=128)

            def load_x_quarter(hf):
                xr = xt_pool.tile([128, NKO, W], F16, tag="xT_r", name="xT_r")
                nc.sync.dma_start(xr[:], xT_v[:, :, ts(hf, W)])
                return xr

            wv_r = load_w("wv", wv, DPC)
            xquart = load_x_quarter(0)
            wq_r = load_w("wq", wq, DPC)
            wk_r = load_w("wk", wk, DPC)

            nc.sync.dma_start(pm_r[:], pmat.ap())
            nc.sync.dma_start(tri_r[:], tri.ap())
            nc.sync.dma_start(cs_t[:], coss.ap())
            nc.sync.dma_start(sn_t[:], sins.ap())
            nc.sync.dma_start(
                wo_r[:], wo.ap().rearrange("(g p) e -> p g e", p=128))

            def do_v(xT_r, hf):
                for tl in range(W // 128):
                    tcN = hf * (W // 128) + tl
                    psv = ps1v.tile([128, DPC], F32, tag="psv", name="psv")
                    for ko in range(NKO):
                        nc.tensor.matmul(psv[:], xT_r[:, ko, ts(tl, 128)],
                                         wv_r[:, ko],
                                         start=(ko == 0), stop=(ko == NKO - 1))
                    nc.vector.tensor_copy(
                        v_aug[:, tcN].rearrange("p (h c) -> p h c",
                                                h=HPC)[:, :, 0:HD],
                        psv[:].rearrange("p (h c) -> p h c", h=HPC))
                    filler()

            def do_qk(xT_r, hf):
                for g in range(NG):
                    psq = ps1qk.tile([128, W], F32, tag="psqk", name="psq")
                    for ko in range(NKO):
                        nc.tensor.matmul(
                            psq[:], wq_r[:, ko, ts(g, 128)], xT_r[:, ko],
                            start=(ko == 0), stop=(ko == NKO - 1))
                    rawq = qkv_pool.tile([128, W], F16, tag="rawq",
                                         name="rawq")
                    nc.scalar.copy(rawq[:], psq[:])
                    psk = ps1qk.tile([128, W], F32, tag="psqk", name="psk")
                    for ko in range(NKO):
                        nc.tensor.matmul(
                            psk[:], wk_r[:, ko, ts(g, 128)], xT_r[:, ko],
                            start=(ko == 0), stop=(ko == NKO - 1))
                    rawk = qkv_pool.tile([128, W], F16, tag="rawk",
                                         name="rawk")
                    nc.scalar.copy(rawk[:], psk[:])
                    for nm, raw, dst in (("q", rawq, qT[g]),
                                         ("k", rawk, kT[g])):
                        psp = ps1p.tile([128, W], F32, tag="psp", name="psp")
                        nc.tensor.matmul(psp[:], pm_r[:], raw[:],
                                         start=True, stop=True)
                        t1 = qkv_pool.tile([128, W], F16, tag=f"t1{nm}",
                                           name="t1")
                        nc.vector.tensor_tensor(t1[:], raw[:],
                                                cs_t[:, ts(hf, W)], MUL)
                        t2 = qkv_pool.tile([128, W], F16, tag=f"t2{nm}",
                                           name="t2")
                        nc.vector.tensor_tensor(t2[:], psp[:],
                                                sn_t[:, ts(hf, W)], MUL)
                        nc.vector.tensor_tensor(dst[:, ts(hf, W)],
                                                t1[:], t2[:],
                                                mybir.AluOpType.add)
                    filler()

            for hf in range(4):
                xT_r = xquart
                if hf < 3:
                    xquart = load_x_quarter(hf + 1)
                    do_v(xT_r, hf)
                    do_qk(xT_r, hf)
                else:
                    do_qk(xT_r, hf)
                    do_v(xT_r, hf)
                if hf == 0:
                    tasks.extend((0, kb, g, h) for kb in range(4)
                                 for g in range(NG) for h in range(2))
                elif hf == 1:
                    tasks.extend((1, kb, g, h) for kb in range(8)
                                 for g in range(NG) for h in range(2))
            while tasks:
                emit_unit(*tasks.pop(0))

        # ---- phase 2: attention (+ interleaved output projection) -----
        with nc.named_scope("attn"), \
             tc.tile_pool(name="att", bufs=16) as att_pool, \
             tc.tile_pool(name="norm", bufs=3) as norm_pool, \
             tc.tile_pool(name="ps2", bufs=2, space="PSUM") as ps2, \
             tc.tile_pool(name="ps2av", bufs=1, space="PSUM") as ps2av:
            pending = []

            def emit_oproj(tcN):
                ysb = norm_pool.tile([128, D_MODEL], F16, tag="ysb",
                                     name="ysb")
                for e2 in range(2):
                    psy = ps2.tile([128, W], F32, tag="sc", name="psy")
                    for g in range(NG):
                        nc.tensor.matmul(psy[:], out_cT[g][:, ts(tcN, 128)],
                                         wo_r[:, g, ts(e2, W)],
                                         start=(g == 0), stop=(g == NG - 1),
                                         skip_group_check=True)
                    nc.vector.tensor_copy(ysb[:, ts(e2, W)], psy[:])
                nc.sync.dma_start(y.ap()[ts(tcN, 128), :], ysb[:])

            def make_entry(qb, kb):
                """scores + exp (+tri) for one kb of qb (two-head tiles)."""
                cs0 = max(0, kb * 128 - qb * W)
                diag = kb * 128 >= qb * W
                atts = []
                for g in range(NG):
                    sc = ps2.tile([128, 2 * W], F32, tag="sc", name="sc")
                    for h in range(2):
                        nc.tensor.matmul(
                            sc[:, h * W + cs0:(h + 1) * W],
                            kT[g][ts(h, HD), ts(kb, 128)],
                            qT[g][ts(h, HD), qb * W + cs0:(qb + 1) * W],
                            start=True, stop=True, skip_group_check=True)
                    att = att_pool.tile([128, 2 * W], F16, tag="attw",
                                        name="att")
                    scv = sc[:].rearrange("p (h w) -> p h w", h=2)
                    atv = att[:].rearrange("p (h w) -> p h w", h=2)
                    nc.scalar.activation(atv[:, :, cs0:], scv[:, :, cs0:],
                                         AF.Exp, scale=SCALE)
                    if diag:
                        for h in range(2):
                            dslc = slice(h * W + cs0, h * W + cs0 + 128)
                            nc.vector.tensor_tensor(att[:, dslc], att[:, dslc],
                                                    tri_r[:], MUL)
                    atts.append(att)
                return (kb, cs0, atts)

            def normalize(av, qb):
                rss = []
                for hh in range(4):
                    rs = norm_pool.tile([1, W], F32, tag=f"rs{hh}", name="rs")
                    nc.scalar.copy(rs[:], av[hh][HD:HD + 1, :])
                    rss.append(rs)
                recs = []
                for hh in range(4):
                    rec = norm_pool.tile([1, W], F32, tag=f"rec{hh}",
                                         name="rec")
                    nc.vector.reciprocal_approx_fast(rec[:], rss[hh][:])
                    recs.append(rec)
                rbs = []
                for hh in range(4):
                    rb = norm_pool.tile([HD, W], F32, tag=f"rb{hh}", name="rb")
                    nc.gpsimd.partition_broadcast(rb[:], recs[hh][:])
                    rbs.append(rb)
                for hh in range(4):
                    g, h = divmod(hh, 2)
                    nc.vector.tensor_tensor(
                        out_cT[g][ts(h, HD), ts(qb, W)],
                        av[hh][0:HD, :], rbs[hh][:], MUL)

            early = []       # pre-built (kb, cs0, atts) entries for qb=2
            carry = []       # pre-built entries for qb=3
            NEARLY = 6
            LAG = 2

            for qb in range(S // W):
                av = [ps2av.tile([HD + 1, W], F32, tag=f"av{hh}",
                                 name=f"av{hh}") for hh in range(4)]
                nkb = (qb + 1) * (W // 128)

                if qb < 2:
                    # AV prologue from stored att01 tiles (PE-only), with
                    # early qb2 score/exp units to keep Scalar busy.
                    for kb in range(nkb):
                        if kb % 2 == 0 and len(early) < NEARLY:
                            early.append(make_entry(2, len(early)))
                        cs0 = max(0, kb * 128 - qb * W)
                        for g in range(NG):
                            for h in range(2):
                                hh = 2 * g + h
                                nc.tensor.matmul(
                                    av[hh][:, cs0:],
                                    v_aug[:, kb, hh * (HD + 1):
                                          (hh + 1) * (HD + 1)],
                                    att01[(qb, kb, g, h)][:, cs0:],
                                    start=(kb == 0), stop=(kb == nkb - 1),
                                    skip_group_check=True)
                        if kb >= 3 and pending:
                            emit_oproj(pending.pop(0))
                    normalize(av, qb)
                    pending.extend(qb * (W // 128) + tl
                                   for tl in range(W // 128))
                    continue

                attq = list(early) if qb == 2 else list(carry)
                early = []
                start_kb = len(attq)

                def emit_av(entry, nkb=nkb, av=av):
                    kb, cs0, atts = entry
                    for g in range(NG):
                        for h in range(2):
                            hh = 2 * g + h
                            nc.tensor.matmul(
                                av[hh][:, cs0:],
                                v_aug[:, kb, hh * (HD + 1):
                                      (hh + 1) * (HD + 1)],
                                atts[g][:, h * W + cs0:(h + 1) * W],
                                start=(kb == 0), stop=(kb == nkb - 1),
                                skip_group_check=True)

                for kb in range(start_kb, nkb):
                    if kb >= 3 and pending:
                        emit_oproj(pending.pop(0))
                    attq.append(make_entry(qb, kb))
                    if qb == 2 and kb >= nkb - 2:
                        carry.append(make_entry(3, kb - (nkb - 2)))
                    while len(attq) > LAG:
                        emit_av(attq.pop(0))
                while attq:
                    emit_av(attq.pop(0))
                normalize(av, qb)
                pending.extend(qb * (W // 128) + tl for tl in range(W // 128))
            for tcN in pending:
                emit_oproj(tcN)

    nc.compile()
    return nc


def _host_inputs():
    d = HD
    inv_freq = THETA ** (-np.arange(0, d, 2, dtype=np.float64) / d)  # [32]
    t = np.arange(S, dtype=np.float64)
    ang = t[None, :] * inv_freq[:, None]          # [32, S]
    C64 = np.repeat(np.cos(ang), 2, axis=0)       # [64, S] per-dim cos
    S64 = np.repeat(np.sin(ang), 2, axis=0).copy()
    S64[0::2] *= -1.0                             # even dims: -sin
    C = np.tile(C64, (2, 1)).astype(np.float16)   # [128, S] two heads
    Sg = np.tile(S64, (2, 1)).astype(np.float16)

    P = np.zeros((128, 128), np.float16)
    idx = np.arange(128)
    P[idx ^ 1, idx] = 1.0

    # tri[k, q] = 1 where q >= k (causal keep), applied post-exp
    T = (np.arange(128)[None, :] >= np.arange(128)[:, None]
         ).astype(np.float16)
    return C, Sg, P, T


def kernel(x, Wq, Wk, Wv, Wo):
    from concourse.bass_utils import run_bass_kernel_spmd

    x = np.asarray(x, np.float32)
    Wq = np.asarray(Wq, np.float32)
    Wk = np.asarray(Wk, np.float32)
    Wv = np.asarray(Wv, np.float32)
    Wo = np.asarray(Wo, np.float32)
    B = x.shape[0]

    if "nc" not in _CACHE:
        _CACHE["nc"] = _build_nc()
    nc = _CACHE["nc"]

    C, Sg, P, T = _host_inputs()
    xTb = [np.ascontiguousarray(x[b].T).astype(np.float16) for b in range(B)]
    in_maps = []
    for c in range(8):
        b, hq = divmod(c, 4)
        sl = slice(hq * DPC, (hq + 1) * DPC)
        in_maps.append({
            "xT": xTb[b],
            "wq": np.ascontiguousarray(Wq[sl, :].T).astype(np.float16),
            "wk": np.ascontiguousarray(Wk[sl, :].T).astype(np.float16),
            "wv": np.ascontiguousarray(Wv[sl, :].T).astype(np.float16),
            "wo": np.ascontiguousarray(Wo[:, sl].T).astype(np.float16),
            "coss": C, "sins": Sg, "pmat": P, "tri": T,
        })

    res = run_bass_kernel_spmd(nc, in_maps, list(range(8)),
                               **_CACHE.get("runkw", {}))
    _CACHE["last_res"] = res
    out = np.zeros((B, S, D_MODEL), np.float32)
    for c in range(8):
        b = c // 4
        out[b] += res.results[c]["y"].astype(np.float32)
    return out
